# revision 13
# baseline (speedup 1.0000x reference)
"""Trainium2 Bass kernel for nn_MetaLayer3 (GNN message passing, 8-core SPMD).

Sharding: position slices (edges/nodes/faces) per core; gather tables (x,
edge_attr, face) replicated in each core's HBM. Random-row access via
indirect DMA (128 rows/instruction). Segment sums: host-sorts streams by
destination, device gathers sorted rows and reduces them with selection-matrix
matmuls accumulated in PSUM. MLPs: PE transposes + PSUM-accumulated block
matmuls (concat never materialized). Global update: per-graph partial sums on
device (piggybacked on tiles already loaded), tiny final MLP on host.
"""
import sys

sys.path.insert(0, '/opt/trn_rl_repo')

import numpy as np

NCORES = 8
N, E, F, G, NF, D = 100000, 400000, 50000, 100, 200000, 128


# ---------------------------------------------------------------- host prep
def _pad_to(a, n, fill=0):
    if len(a) < n:
        a = np.concatenate([a, np.full(n - len(a), fill, a.dtype)])
    return a


def _cols128(a, ntiles, fill=0):
    """[K] -> [128, ntiles]; column t holds a[t*128:(t+1)*128] (padded)."""
    a = _pad_to(a, ntiles * 128, fill)
    return np.ascontiguousarray(a.reshape(ntiles, 128).T)


class _Stream:
    """Dest-sorted gather stream feeding 512-dest PSUM blocks."""

    def __init__(self, dest, src, n_dest):
        order = np.argsort(dest, kind='stable')
        self.dest = dest[order].astype(np.int64)
        self.src = src[order].astype(np.int64)
        self.nblk = -(-n_dest // 512)
        self.bnds = np.searchsorted(self.dest, np.arange(self.nblk + 1) * 512)

    def max_tiles(self):
        cnt = np.diff(self.bnds)
        return max(1, int(np.max(-(-cnt // 128))))

    def layout(self, tpb):
        """[128, nblk*tpb] int32 src + f32 dstrel (block-relative, pad=-1)."""
        srcs = np.zeros((128, self.nblk * tpb), np.int32)
        rels = np.full((128, self.nblk * tpb), -1.0, np.float32)
        for b in range(self.nblk):
            i0, i1 = self.bnds[b], self.bnds[b + 1]
            n = i1 - i0
            nt = -(-n // 128) if n else 0
            assert nt <= tpb
            if n == 0:
                continue
            s = _cols128(self.src[i0:i1].astype(np.int32), nt)
            r = _cols128((self.dest[i0:i1] - b * 512).astype(np.float32), nt, fill=-1.0)
            srcs[:, b * tpb:b * tpb + nt] = s
            rels[:, b * tpb:b * tpb + nt] = r
        return srcs, rels


def _selg(batch, t0, t1, nchunk):
    """One-hot [128, nchunk*512] f32: S[g, t] = (batch[t0+t] == g)."""
    g = batch[t0:t1].astype(np.int64)
    g = np.concatenate([g, np.full(nchunk * 512 - len(g), -1, np.int64)])
    S = np.zeros((128, nchunk * 512), np.float32)
    v = np.nonzero(g >= 0)[0]
    S[g[v], v] = 1.0
    return S


def _prep_device(d, ix, dims):
    E_ = dims['E']
    ED, ND, FD = dims['ED'], dims['ND'], dims['FD']
    ECH, NCH, FCH = dims['ECH'], dims['NCH'], dims['FCH']
    row, col = ix['edge_index'][0], ix['edge_index'][1]
    f0, f1 = ix['face_index'][0], ix['face_index'][1]
    nfn, nff = ix['nf_node'], ix['nf_face']
    e0, e1 = d * ED, (d + 1) * ED
    n0, n1 = d * ND, (d + 1) * ND
    c0, c1 = d * FD, (d + 1) * FD

    o = {}
    o['eg_xr'] = _cols128(row[e0:e1].astype(np.int32), 4 * ECH)
    o['eg_xc'] = _cols128(col[e0:e1].astype(np.int32), 4 * ECH)
    o['eg_f0'] = _cols128(f0[e0:e1].astype(np.int32), 4 * ECH)
    o['eg_f1'] = _cols128(f1[e0:e1].astype(np.int32), 4 * ECH)

    def mk(destg, srcg, lo, hi, nd):
        m = (destg >= lo) & (destg < hi)
        return _Stream(destg[m] - lo, srcg[m], nd)

    aE = np.arange(E_)
    o['streams'] = {
        'n_row': mk(row, aE, n0, n1, ND),
        'n_col': mk(col, aE, n0, n1, ND),
        'n_nf': mk(nfn, nff, n0, n1, ND),
        'f_f0': mk(f0, aE, c0, c1, FD),
        'f_f1': mk(f1, aE, c0, c1, FD),
        'f_nf': mk(nff, nfn, c0, c1, FD),
    }

    o['selg_e'] = _selg(ix['edge_batch'], e0, e1, ECH)
    o['selg_n'] = _selg(ix['node_batch'], n0, n1, NCH)
    o['selg_f'] = _selg(ix['face_batch'], c0, c1, FCH)
    # per-tile graph ids (f32, pad -1) for glob piggyback Sel builds
    o['gid_e'] = _cols128(ix['edge_batch'][e0:e1].astype(np.float32), 4 * ECH, fill=-1.0)
    o['gid_n'] = _cols128(ix['node_batch'][n0:n1].astype(np.float32), 4 * NCH, fill=-1.0)
    o['gid_f'] = _cols128(ix['face_batch'][c0:c1].astype(np.float32), 4 * FCH, fill=-1.0)

    fm = 1.0 - ix['face_mask'][c0:c1].astype(np.float32)
    o['fmask'] = _cols128(fm, 4 * FCH, fill=0.0)
    return o


# ---------------------------------------------------------------- device program
def _build(dims, tpbs, mm_fast=False):
    import concourse.bass as bass
    import concourse.mybir as mybir
    import concourse.tile as tile
    from concourse import bacc
    from concourse.masks import make_identity
    import contextlib

    f32 = mybir.dt.float32
    r = mybir.dt.float32r if mm_fast else f32
    ED, ND, FD = dims['ED'], dims['ND'], dims['FD']
    ECH, NCH, FCH = dims['ECH'], dims['NCH'], dims['FCH']
    N_, E_, F_, G_ = dims['N'], dims['E'], dims['F'], dims['G']

    nc = bacc.Bacc("TRN2", target_bir_lowering=False, debug=False)

    Xt = nc.dram_tensor("x", [N_, D], f32, kind="ExternalInput").ap()
    EAt = nc.dram_tensor("ea", [E_, D], f32, kind="ExternalInput").ap()
    FCt = nc.dram_tensor("face", [F_, D], f32, kind="ExternalInput").ap()
    Ut = nc.dram_tensor("u", [G_, D], f32, kind="ExternalInput").ap()
    Xo = nc.dram_tensor("x_own", [ND, D], f32, kind="ExternalInput").ap()
    EAo = nc.dram_tensor("ea_own", [ED, D], f32, kind="ExternalInput").ap()
    FCo = nc.dram_tensor("face_own", [FD, D], f32, kind="ExternalInput").ap()

    Wdef = [("edge", 6), ("node", 5), ("facem", 5)]
    W = {}
    for nm, nb in Wdef:
        W[nm, 1] = nc.dram_tensor(f"{nm}_W1", [nb * D, D], f32, kind="ExternalInput").ap()
        W[nm, 2] = nc.dram_tensor(f"{nm}_W2", [D, D], f32, kind="ExternalInput").ap()
        W[nm, 'b1'] = nc.dram_tensor(f"{nm}_b1", [1, D], f32, kind="ExternalInput").ap()
        W[nm, 'b2'] = nc.dram_tensor(f"{nm}_b2", [1, D], f32, kind="ExternalInput").ap()

    IX = {}
    for k in ('eg_xr', 'eg_xc', 'eg_f0', 'eg_f1'):
        IX[k] = nc.dram_tensor(k, [128, 4 * ECH], mybir.dt.int32, kind="ExternalInput").ap()
    snames = ('n_row', 'n_col', 'n_nf', 'f_f0', 'f_f1', 'f_nf')
    sblk = {'n_row': NCH, 'n_col': NCH, 'n_nf': NCH, 'f_f0': FCH, 'f_f1': FCH, 'f_nf': FCH}
    for s in snames:
        ncols = sblk[s] * tpbs[s]
        IX[s + '_src'] = nc.dram_tensor(s + "_src", [128, ncols], mybir.dt.int32, kind="ExternalInput").ap()
        IX[s + '_rel'] = nc.dram_tensor(s + "_rel", [128, ncols], f32, kind="ExternalInput").ap()
    for k, nch in (('selg_e', ECH), ('selg_n', NCH), ('selg_f', FCH)):
        IX[k] = nc.dram_tensor(k, [128, nch * 512], f32, kind="ExternalInput").ap()
    for k, nt in (('gid_e', 4 * ECH), ('gid_n', 4 * NCH), ('gid_f', 4 * FCH)):
        IX[k] = nc.dram_tensor(k, [128, nt], f32, kind="ExternalInput").ap()
    IX['fmask'] = nc.dram_tensor("fmask", [128, 4 * FCH], f32, kind="ExternalInput").ap()

    ENew = nc.dram_tensor("edge_new", [ED, D], f32, kind="ExternalOutput").ap()
    XNew = nc.dram_tensor("x_new", [ND, D], f32, kind="ExternalOutput").ap()
    FNew = nc.dram_tensor("face_new", [FD, D], f32, kind="ExternalOutput").ap()
    GPart = nc.dram_tensor("glob_part", [3 * 128, G_], f32, kind="ExternalOutput").ap()

    tables = {'x': Xt, 'ea': EAt, 'face': FCt}
    stbl = {'n_row': 'ea', 'n_col': 'ea', 'n_nf': 'face', 'f_f0': 'ea', 'f_f1': 'ea', 'f_nf': 'x'}

    with tile.TileContext(nc) as tc:
        ctx = contextlib.ExitStack()
        with ctx:
            con = ctx.enter_context(tc.tile_pool(name="con", bufs=1))
            gat = ctx.enter_context(tc.tile_pool(name="gat", bufs=12))
            gtp = ctx.enter_context(tc.tile_pool(name="gtp", bufs=3))
            hsb = ctx.enter_context(tc.tile_pool(name="hsb", bufs=3))
            osb = ctx.enter_context(tc.tile_pool(name="osb", bufs=4))
            selp = ctx.enter_context(tc.tile_pool(name="selp", bufs=6))
            ldp = ctx.enter_context(tc.tile_pool(name="ldp", bufs=6))
            sgp = ctx.enter_context(tc.tile_pool(name="sgp", bufs=2))
            hp = ctx.enter_context(tc.tile_pool(name="hp", bufs=2, space="PSUM"))
            segp = ctx.enter_context(tc.tile_pool(name="segp", bufs=2, space="PSUM"))
            pp = ctx.enter_context(tc.tile_pool(name="pp", bufs=3, space="PSUM"))
            gap = ctx.enter_context(tc.tile_pool(name="gap", bufs=1, space="PSUM"))

            ident = con.tile([128, 128], f32, tag="ident", name="ident")
            make_identity(nc, ident[:])
            iota512 = con.tile([128, 512], f32, tag="iota", name="iota512")
            nc.gpsimd.iota(iota512[:], pattern=[[1, 512]], base=0,
                           channel_multiplier=0, allow_small_or_imprecise_dtypes=True)

            WS = {}
            for nm, nb in Wdef:
                for b in range(nb):
                    t = con.tile([128, D], f32, tag=f"w1_{nm}_{b}", name=f"w1_{nm}_{b}")
                    nc.sync.dma_start(out=t[:], in_=W[nm, 1][b * D:(b + 1) * D, :])
                    WS[nm, 1, b] = t
                t = con.tile([128, D], f32, tag=f"w2_{nm}", name=f"w2_{nm}")
                nc.sync.dma_start(out=t[:], in_=W[nm, 2][:, :])
                WS[nm, 2] = t
                tb = con.tile([1, D], f32, tag=f"b1_{nm}", name=f"b1_{nm}")
                nc.sync.dma_start(out=tb[:], in_=W[nm, 'b1'][:, :])
                WS[nm, 'b1'] = tb
                tb2 = con.tile([1, D], f32, tag=f"b2_{nm}", name=f"b2_{nm}")
                nc.sync.dma_start(out=tb2[:], in_=W[nm, 'b2'][:, :])
                WS[nm, 'b2'] = tb2
            ones_row = con.tile([1, 128], f32, tag="ones", name="ones_row")
            nc.vector.memset(ones_row[:], 1.0)
            for nm, _ in Wdef:
                ps = pp.tile([128, 128], f32, tag="ps")
                nc.tensor.transpose(out=ps[:, :1], in_=WS[nm, 'b1'][:1, :], identity=ident[:1, :1])
                t = con.tile([128, 1], f32, tag=f"b1c_{nm}", name=f"b1c_{nm}")
                nc.vector.tensor_copy(out=t[:], in_=ps[:, :1])
                WS[nm, 'b1c'] = t
                # b2 broadcast to all partitions via K=1 matmul: ones.T @ b2_row
                ps2 = pp.tile([128, 128], f32, tag="ps")
                nc.tensor.matmul(ps2[:], lhsT=ones_row[:1, :], rhs=WS[nm, 'b2'][:1, :],
                                 start=True, stop=True)
                b2x = con.tile([128, D], f32, tag=f"b2x_{nm}", name=f"b2x_{nm}")
                nc.vector.tensor_copy(out=b2x[:], in_=ps2[:])
                WS[nm, 'b2x'] = b2x

            # U1T[upd] = u @ W1_ublock  ([G,128] on partitions 0..G-1)
            u_sb = con.tile([128, D], f32, tag="u_sb", name="u_sb")
            nc.sync.dma_start(out=u_sb[:G_, :], in_=Ut[:, :])
            ups = pp.tile([128, 128], f32, tag="ps")
            nc.tensor.transpose(out=ups[:, :G_], in_=u_sb[:G_, :], identity=ident[:G_, :G_])
            uT = con.tile([128, G_], f32, tag="uT", name="uT")
            nc.vector.tensor_copy(out=uT[:], in_=ups[:, :G_])
            U1T = {}
            for nm, bidx in (('edge', 3), ('node', 3), ('facem', 3)):
                ps = pp.tile([128, 128], f32, tag="ps")
                nc.tensor.matmul(ps[:G_, :], lhsT=uT[:, :].bitcast(r),
                                 rhs=WS[nm, 1, bidx][:].bitcast(r), start=True, stop=True)
                t = con.tile([128, D], f32, tag=f"u1t_{nm}", name=f"u1t_{nm}")
                nc.vector.memset(t[:], 0.0)
                nc.vector.tensor_copy(out=t[:G_, :], in_=ps[:G_, :])
                U1T[nm] = t

            IXS = {}
            for k, ap in IX.items():
                if k.startswith('selg'):
                    continue  # too big for SBUF; streamed per chunk
                t = con.tile([128, ap.shape[1]], ap.dtype, tag=f"ix_{k}", name=f"ix_{k}")
                nc.sync.dma_start(out=t[:], in_=ap[:, :])
                IXS[k] = t

            gacc = gap.tile([128, 384], f32, tag="gacc", name="gacc")  # gx|gea|gfc at 0/128/256

            def gather128(tbl, idxcol):
                g = gat.tile([128, D], f32, tag="g")
                nc.gpsimd.indirect_dma_start(
                    out=g[:], out_offset=None, in_=tables[tbl][:],
                    in_offset=bass.IndirectOffsetOnAxis(ap=idxcol, axis=0))
                return g

            def transpose_to(gt_tile, j, src):
                ps = pp.tile([128, 128], f32, tag="ps")
                nc.tensor.transpose(out=ps[:], in_=src[:], identity=ident[:])
                nc.any.tensor_copy(out=gt_tile[:, j * 128:(j + 1) * 128], in_=ps[:])

            def glob_piggy(src, gidcol, colbase, start, stop):
                sel = selp.tile([128, 512], f32, tag="sel")
                nc.vector.tensor_tensor(out=sel[:, :G_], in0=gidcol.to_broadcast([128, G_]),
                                        in1=iota512[:, :G_], op=mybir.AluOpType.is_equal)
                nc.tensor.matmul(gacc[:, colbase:colbase + G_], lhsT=src[:].bitcast(r),
                                 rhs=sel[:, :G_].bitcast(r), start=start, stop=stop,
                                 skip_group_check=True)

            def mlp_tail(nm, hpsum, selg_dram, chunk, nvalid, out_dram, fmask_cols=None):
                sg = sgp.tile([128, 512], f32, tag="sg")
                nc.sync.dma_start(out=sg[:], in_=selg_dram[:, chunk * 512:(chunk + 1) * 512])
                nc.tensor.matmul(hpsum[:], lhsT=U1T[nm][:G_, :].bitcast(r),
                                 rhs=sg[:G_, :].bitcast(r), start=False, stop=True)
                h = hsb.tile([128, 512], f32, tag="h")
                nc.scalar.activation(h[:], hpsum[:], mybir.ActivationFunctionType.Relu,
                                     bias=WS[nm, 'b1c'][:, :1])
                for j in range(4):
                    tok0 = chunk * 512 + j * 128
                    nv = min(128, nvalid - tok0)
                    if nv <= 0:
                        break
                    ops = pp.tile([128, 128], f32, tag="ps")
                    nc.tensor.matmul(ops[:], lhsT=h[:, j * 128:(j + 1) * 128].bitcast(r),
                                     rhs=WS[nm, 2][:].bitcast(r), start=True, stop=True)
                    ot = osb.tile([128, 128], f32, tag="ot")
                    nc.vector.tensor_add(out=ot[:], in0=ops[:], in1=WS[nm, 'b2x'][:])
                    if fmask_cols is not None:
                        nc.vector.tensor_tensor(
                            out=ot[:], in0=ot[:],
                            in1=fmask_cols[:, chunk * 4 + j:chunk * 4 + j + 1].to_broadcast([128, 128]),
                            op=mybir.AluOpType.mult)
                    nc.sync.dma_start(out=out_dram[tok0:tok0 + nv, :], in_=ot[:nv, :])

            # ---------------- EDGE UPDATE ----------------
            eblocks = [('ea', None, 0), ('xr', 'eg_xr', 1), ('xc', 'eg_xc', 2),
                       ('f0', 'eg_f0', 4), ('f1', 'eg_f1', 5)]
            for c in range(ECH):
                hpsum = hp.tile([128, 512], f32, tag="hps")
                for bi, (bname, ixk, wb) in enumerate(eblocks):
                    gt = gtp.tile([128, 512], f32, tag="gt")
                    for j in range(4):
                        ti = c * 4 + j
                        if ixk is None:
                            src = ldp.tile([128, 128], f32, tag="ea")
                            e0 = ti * 128
                            nv = min(128, ED - e0)
                            if nv < 128:
                                nc.vector.memset(src[:], 0.0)
                            if nv > 0:
                                nc.sync.dma_start(out=src[:nv, :], in_=EAo[e0:e0 + nv, :])
                            glob_piggy(src, IXS['gid_e'][:, ti:ti + 1], 128,
                                       start=(ti == 0), stop=(ti == 4 * ECH - 1))
                        else:
                            src = gather128('x' if bname in ('xr', 'xc') else 'face',
                                            IXS[ixk][:, ti:ti + 1])
                        transpose_to(gt, j, src)
                    nc.tensor.matmul(hpsum[:], lhsT=WS['edge', 1, wb][:].bitcast(r),
                                     rhs=gt[:].bitcast(r), start=(bi == 0), stop=False)
                mlp_tail('edge', hpsum, IX['selg_e'], c, ED, ENew)

            # ---------------- NODE / FACE UPDATES ----------------
            for nm, nblk, ndst, outd, ownt, strm3, selgk, gidk, gcol, fmk in (
                ('node', NCH, ND, XNew, Xo, ('n_row', 'n_col', 'n_nf'), 'selg_n', 'gid_n', 0, None),
                ('facem', FCH, FD, FNew, FCo, ('f_f0', 'f_f1', 'f_nf'), 'selg_f', 'gid_f', 256, 'fmask'),
            ):
                for c in range(nblk):
                    hpsum = hp.tile([128, 512], f32, tag="hps")
                    gt = gtp.tile([128, 512], f32, tag="gt")
                    for j in range(4):
                        ti = c * 4 + j
                        src = ldp.tile([128, 128], f32, tag="own")
                        t0 = ti * 128
                        nv = min(128, ndst - t0)
                        if nv < 128:
                            nc.vector.memset(src[:], 0.0)
                        if nv > 0:
                            nc.sync.dma_start(out=src[:nv, :], in_=ownt[t0:t0 + nv, :])
                        glob_piggy(src, IXS[gidk][:, ti:ti + 1], gcol,
                                   start=(ti == 0), stop=(ti == 4 * nblk - 1))
                        transpose_to(gt, j, src)
                    nc.tensor.matmul(hpsum[:], lhsT=WS[nm, 1, 0][:].bitcast(r),
                                     rhs=gt[:].bitcast(r), start=True, stop=False)
                    for si, sname in enumerate(strm3):
                        tpb = tpbs[sname]
                        seg = segp.tile([128, 512], f32, tag="seg")
                        for t in range(tpb):
                            gi = c * tpb + t
                            g = gather128(stbl[sname], IXS[sname + '_src'][:, gi:gi + 1])
                            sel = selp.tile([128, 512], f32, tag="sel")
                            nc.vector.tensor_tensor(
                                out=sel[:], in0=IXS[sname + '_rel'][:, gi:gi + 1].to_broadcast([128, 512]),
                                in1=iota512[:], op=mybir.AluOpType.is_equal)
                            nc.tensor.matmul(seg[:], lhsT=g[:].bitcast(r), rhs=sel[:].bitcast(r),
                                             start=(t == 0), stop=(t == tpb - 1))
                        segsb = hsb.tile([128, 512], f32, tag="segsb")
                        nc.any.tensor_copy(out=segsb[:], in_=seg[:])
                        wb = (1, 2, 4)[si]
                        nc.tensor.matmul(hpsum[:], lhsT=WS[nm, 1, wb][:].bitcast(r),
                                         rhs=segsb[:].bitcast(r), start=False, stop=False)
                    mlp_tail(nm, hpsum, IX[selgk], c, ndst, outd,
                             fmask_cols=IXS[fmk] if fmk else None)

            for gi, colbase in ((0, 0), (1, 128), (2, 256)):
                t = osb.tile([128, G_], f32, tag="gout")
                nc.vector.tensor_copy(out=t[:], in_=gacc[:, colbase:colbase + G_])
                nc.sync.dma_start(out=GPart[gi * 128:(gi + 1) * 128, :], in_=t[:])

    nc.compile()
    return nc


# ---------------------------------------------------------------- entry point
def _mlp_np(h, W1, b1, W2, b2):
    return np.maximum(h @ W1 + b1, 0.0) @ W2 + b2


def _prepare(inputs, dims):
    ix = {k: np.asarray(inputs[k]) for k in
          ('edge_index', 'face_index', 'nf_node', 'nf_face',
           'node_batch', 'edge_batch', 'face_batch', 'face_mask')}
    preps = [_prep_device(d, ix, dims) for d in range(NCORES)]

    snames = ('n_row', 'n_col', 'n_nf', 'f_f0', 'f_f1', 'f_nf')
    tpbs = {s: max(p['streams'][s].max_tiles() for p in preps) for s in snames}

    nc = _build(dims, tpbs)

    f32 = np.float32
    x = np.ascontiguousarray(inputs['x'], f32)
    ea = np.ascontiguousarray(inputs['edge_attr'], f32)
    fc = np.ascontiguousarray(inputs['face'], f32)
    u = np.ascontiguousarray(inputs['u'], f32)

    in_maps = []
    for d in range(NCORES):
        p = preps[d]
        m = {'x': x, 'ea': ea, 'face': fc, 'u': u,
             'x_own': x[d * dims['ND']:(d + 1) * dims['ND']],
             'ea_own': ea[d * dims['ED']:(d + 1) * dims['ED']],
             'face_own': fc[d * dims['FD']:(d + 1) * dims['FD']]}
        for nm in ('edge', 'node', 'facem'):
            m[f'{nm}_W1'] = np.ascontiguousarray(inputs[f'{nm}_W1'], f32)
            m[f'{nm}_W2'] = np.ascontiguousarray(inputs[f'{nm}_W2'], f32)
            m[f'{nm}_b1'] = np.ascontiguousarray(inputs[f'{nm}_b1'], f32).reshape(1, D)
            m[f'{nm}_b2'] = np.ascontiguousarray(inputs[f'{nm}_b2'], f32).reshape(1, D)
        for k in ('eg_xr', 'eg_xc', 'eg_f0', 'eg_f1', 'selg_e', 'selg_n', 'selg_f',
                  'gid_e', 'gid_n', 'gid_f', 'fmask'):
            m[k] = p[k]
        for s in snames:
            src, rel = p['streams'][s].layout(tpbs[s])
            m[s + '_src'] = src
            m[s + '_rel'] = rel
        in_maps.append(m)
    return nc, in_maps


def kernel(**inputs):
    from concourse.bass_utils import run_bass_kernel_spmd

    dims = dict(N=N, E=E, F=F, G=G, NF=NF,
                ED=E // NCORES, ND=N // NCORES, FD=F // NCORES)
    dims['ECH'] = -(-dims['ED'] // 512)
    dims['NCH'] = -(-dims['ND'] // 512)
    dims['FCH'] = -(-dims['FD'] // 512)

    nc, in_maps = _prepare(inputs, dims)
    res = run_bass_kernel_spmd(nc, in_maps, core_ids=list(range(NCORES)))
    globals()['_last_res'] = res

    x_new = np.concatenate([res.results[d]['x_new'] for d in range(NCORES)], 0)
    edge_new = np.concatenate([res.results[d]['edge_new'] for d in range(NCORES)], 0)
    face_new = np.concatenate([res.results[d]['face_new'] for d in range(NCORES)], 0)

    u_new = _glob_host(inputs, [res.results[d]['glob_part'] for d in range(NCORES)])
    return (x_new, edge_new, u_new, face_new)


def _glob_host(inputs, gparts):
    f32 = np.float32
    u = np.asarray(inputs['u'], f32)
    gp = np.sum(gparts, axis=0)
    seg_x, seg_ea, seg_fc = gp[:128].T, gp[128:256].T, gp[256:384].T  # [G, 128]
    glob_in = np.concatenate([u, seg_x, seg_ea, seg_fc], 1)
    u_new = _mlp_np(glob_in, np.asarray(inputs['glob_W1'], f32),
                    np.asarray(inputs['glob_b1'], f32),
                    np.asarray(inputs['glob_W2'], f32),
                    np.asarray(inputs['glob_b2'], f32))
    return u_new.astype(f32)
